# revision 13
# baseline (speedup 1.0000x reference)
"""Multi-head attention (16 heads, d_model=1024, B=2, T=S=2048) on 8 trn2 cores.

Strategy v4: shard by (batch, head-quad) — core c handles batch c//4, heads
4*(c%4)..+4. Differences vs v3 (the 257us baseline):
  - attn_bias enters ADDITIVELY as raw fp16: the PSUM->SBUF drain of the
    score tile is one DVE scalar_tensor_tensor (sc*0.125 + bias) into an
    f32 SBUF slab, replacing the separate exp(bias) bf16 multiply.
  - exp runs as wide [128,4096] SBUF->SBUF activations (4 s-tiles per
    ACTIVATE) cutting ScalarE per-instruction overhead ~25%.
  - t-chunks of 256 (not 512) so the 4 per-head ctx accumulators fit in
    2 PSUM banks and can double-buffer across t-chunks: no end-of-chunk
    normalization stall, PE stays HAM-warm.
  - all DMA sources are host-relaid so every transfer is >=2KB/partition
    contiguous.
  - Q-projection drain runs on ScalarE (Identity+bias), normalization uses
    reciprocal straight from PSUM; denominators still ride the V ones-column.
Host sums the 4 partials per batch and adds bo + bv@Wo.T.
"""

import sys

sys.path.insert(0, "/opt/trn_rl_repo")

from collections import deque
from contextlib import ExitStack

import ml_dtypes
import numpy as np

from concourse import bacc, mybir
from concourse.bass import ts
from concourse.bass_utils import run_bass_kernel_spmd
from concourse.tile import TileContext
from concourse.tile_rust import add_dep_helper

B, T, S, D, H, HD = 2, 2048, 2048, 1024, 16, 64
NCORES = 8
HPC = 4  # heads per core
DPC = HPC * HD  # 256 head-dims per core
DCH = D // 128  # 8 dmodel chunks
NST = S // 128  # 16 s-tiles
TCH = 256  # t-chunk
NTCH = T // TCH  # 8 t-chunks
# column-block order within sc/cps/eb/pt tiles: block i holds head HB[i].
# j-major so the row-group-concurrent score pair (j=0,1) lands in different
# PSUM banks (block 2j+hp), and so ctx accumulation start/stop can follow
# first-in-bank/last-in-bank order (bank0 = blocks 0,1; bank1 = blocks 2,3).
HB = [0, 2, 1, 3]
SLAB = 4  # s-tiles per exp slab
EB_PF = 6  # eb DMA prefetch depth (slots)
BF = mybir.dt.bfloat16
F16 = mybir.dt.float16
F32 = mybir.dt.float32
EXP = mybir.ActivationFunctionType.Exp
IDENT = mybir.ActivationFunctionType.Identity
ADD = mybir.AluOpType.add
MULT = mybir.AluOpType.mult

_PROGRAM = None


def build_program():
    nc = bacc.Bacc()
    # host-relaid inputs; all DMAs are contiguous per partition
    q5 = nc.declare_dram_parameter("q5", [4, 128, DCH, 512], BF, isOutput=False)
    k5 = nc.declare_dram_parameter("k5", [4, 128, DCH, 512], BF, isOutput=False)
    v5 = nc.declare_dram_parameter("v5", [NST, 128, DCH, 128], BF, isOutput=False)
    # raw additive bias: eb6[tch, st, p, h*TCH+t'] fp16
    eb6 = nc.declare_dram_parameter(
        "eb6", [NTCH, NST, 128, HPC * TCH], F16, isOutput=False
    )
    wq5 = nc.declare_dram_parameter("wq5", [128, DCH, DPC], BF, isOutput=False)
    wk5 = nc.declare_dram_parameter("wk5", [128, DCH, DPC], BF, isOutput=False)
    wv5 = nc.declare_dram_parameter("wv5", [128, DCH, DPC], BF, isOutput=False)
    wo5 = nc.declare_dram_parameter("wo5", [128, 2, D], BF, isOutput=False)
    bq_d = nc.declare_dram_parameter("bq", [128, 2, 1], F32, isOutput=False)
    outp = nc.declare_dram_parameter("outp", [T, D], BF, isOutput=True)

    with TileContext(nc) as tc, ExitStack() as ctx:
        consts = ctx.enter_context(tc.tile_pool(name="consts", bufs=1))
        ld_pool = ctx.enter_context(tc.tile_pool(name="ld", bufs=2))
        qkv_pool = ctx.enter_context(tc.tile_pool(name="qkv", bufs=1))
        vsb_pool = ctx.enter_context(tc.tile_pool(name="vsb", bufs=1))
        eb_pool = ctx.enter_context(tc.tile_pool(name="eb", bufs=EB_PF + 2))
        pre_pool = ctx.enter_context(tc.tile_pool(name="pre", bufs=2))
        pt_pool = ctx.enter_context(tc.tile_pool(name="pt", bufs=3))
        norm_pool = ctx.enter_context(tc.tile_pool(name="norm", bufs=4))
        outs_pool = ctx.enter_context(tc.tile_pool(name="outs", bufs=3))
        # PSUM: sc tag 2 bufs x 2 banks = 4, ctx 2 tags x 2 banks = 4 -> 8
        ps_pool = ctx.enter_context(tc.tile_pool(name="ps", bufs=2, space="PSUM"))
        ctx_ps = ctx.enter_context(tc.tile_pool(name="ctx_ps", bufs=1, space="PSUM"))

        # preload the exp table while initial DMAs stream
        warm = consts.tile([1, 8], F32, tag="warm")
        nc.vector.memset(warm[:], 0.0)
        nc.scalar.activation(out=warm[:], in_=warm[:], func=EXP)
        z_sb = consts.tile([128, 512], BF, tag="zeros")
        nc.vector.memset(z_sb[:], 0.0)

        # ---- constants ----
        wq_sb = consts.tile([128, DCH, DPC], BF, tag="wq")
        wk_sb = consts.tile([128, DCH, DPC], BF, tag="wk")
        wv_sb = consts.tile([128, DCH, DPC], BF, tag="wv")
        wo_sb = consts.tile([128, 2, D], BF, tag="wo")
        bq_sb = consts.tile([128, 2, 1], F32, tag="bq")
        nc.sync.dma_start(out=bq_sb, in_=bq_d[:])

        def load_late_consts():
            nc.sync.dma_start(out=wk_sb, in_=wk5[:])
            nc.sync.dma_start(out=wv_sb, in_=wv5[:])
            nc.sync.dma_start(out=wo_sb, in_=wo5[:])

        # persistent activations; partition p = j*64+d, axis1 = hp
        QT_sb = qkv_pool.tile([128, 2, T], BF, tag="QT")
        KT_sb = qkv_pool.tile([128, 2, S], BF, tag="KT")
        ctxT_sb = qkv_pool.tile([128, 2, T], BF, tag="ctxT")
        v_tiles = [
            vsb_pool.tile([128, HPC, HD + 1], BF, tag=f"v{st}", name=f"v{st}")
            for st in range(NST)
        ]

        def load_q_chunk(tch2):
            qt_sb = ld_pool.tile([128, DCH, 512], BF, tag="qld", name="qt_sb")
            nc.sync.dma_start(out=qt_sb, in_=q5[tch2])
            return qt_sb

        def load_k_chunk(tch2):
            kt_sb = ld_pool.tile([128, DCH, 512], BF, tag="kld", name="kt_sb")
            nc.sync.dma_start(out=kt_sb, in_=k5[tch2])
            return kt_sb

        def proj_q_half(qt_sb, tch2, half):
            pq = ps_pool.tile([128, 1024], F32, tag="sc", name="pq")
            for c in range(DCH):
                nc.tensor.matmul(
                    pq[:, 0:512],
                    lhsT=wq_sb[:, c, ts(half, 128)],
                    rhs=qt_sb[:, c, :],
                    start=(c == 0),
                    stop=(c == DCH - 1),
                )
            # QT = (Q + bq) / 8 (attention scale folded in; bq is /8 on host)
            nc.scalar.activation(
                out=QT_sb[:, half, ts(tch2, 512)],
                in_=pq[:, 0:512],
                func=IDENT,
                bias=bq_sb[:, half, :],
                scale=0.125,
            )

        def proj_k_half(kt_sb, tch2, half):
            pk = ps_pool.tile([128, 1024], F32, tag="sc", name="pk")
            for c in range(DCH):
                nc.tensor.matmul(
                    pk[:, 0:512],
                    lhsT=wk_sb[:, c, ts(half, 128)],
                    rhs=kt_sb[:, c, :],
                    start=(c == 0),
                    stop=(c == DCH - 1),
                )
            nc.vector.tensor_copy(
                out=KT_sb[:, half, ts(tch2, 512)], in_=pk[:, 0:512]
            )

        def make_q_parts(tch2):
            state = {}

            def part0():
                state["qt"] = load_q_chunk(tch2)
                proj_q_half(state["qt"], tch2, 0)

            def part1():
                proj_q_half(state["qt"], tch2, 1)

            return part0, part1

        def make_k_parts(tch2):
            state = {}

            def part0():
                state["kt"] = load_k_chunk(tch2)
                proj_k_half(state["kt"], tch2, 0)

            def part1():
                proj_k_half(state["kt"], tch2, 1)

            return part0, part1

        def proj_v_tile(st):
            vt_sb = ld_pool.tile([128, DCH, 128], BF, tag="vld", name="vt_sb", bufs=3)
            nc.sync.dma_start(out=vt_sb, in_=v5[st])
            pv = ps_pool.tile([128, 1024], F32, tag="sc", name="pv")
            for c in range(DCH):
                nc.tensor.matmul(
                    pv[:, 0:DPC],
                    lhsT=vt_sb[:, c, :],
                    rhs=wv_sb[:, c, :],
                    start=(c == 0),
                    stop=(c == DCH - 1),
                )
            v_sb = v_tiles[st]
            nc.vector.tensor_copy(
                out=v_sb[:, :, 0:HD],
                in_=pv[:, 0:DPC].rearrange("p (h d) -> p h d", h=HPC),
            )
            nc.vector.memset(v_sb[:, :, HD : HD + 1], 1.0)

        def out_proj_tile(tch, tt):
            # out rows [tch*TCH + tt*128, +128); emitted during the NEXT tch
            t0 = tch * TCH + tt * 128
            po = ps_pool.tile([128, 1024], F32, tag="sc", name="po")
            for eh in range(2):
                for half in range(2):
                    nc.tensor.matmul(
                        po[:, ts(eh, 512)],
                        lhsT=ctxT_sb[:, half, t0 : t0 + 128],
                        rhs=wo_sb[:, half, ts(eh, 512)],
                        start=(half == 0),
                        stop=(half == 1),
                    )
            out_sb = outs_pool.tile([128, D], BF, tag="out", name="out_sb")
            nc.vector.tensor_copy(out=out_sb, in_=po[:])
            nc.sync.dma_start(out=outp[t0 : t0 + 128, :], in_=out_sb)

        def norm_block(cps, tch, i):
            # denom lives in row HD; normalize block i (head HB[i]) into ctxT
            h = HB[i]
            dn = norm_pool.tile([1, TCH], F32, tag="dn", name="dn")
            nc.vector.tensor_copy(out=dn[:], in_=cps[HD : HD + 1, ts(i, TCH)])
            rc = norm_pool.tile([1, TCH], F32, tag="rc", name="rc")
            nc.vector.reciprocal_approx_fast(out=rc[:], in_=dn[:])
            rrep = norm_pool.tile([64, TCH], F32, tag="rrep", name="rrep")
            nc.gpsimd.partition_broadcast(rrep[:], rc[:], channels=64)
            nc.vector.tensor_tensor(
                out=ctxT_sb[ts(h % 2, HD), h // 2, tch * TCH : tch * TCH + TCH],
                in0=cps[0:HD, ts(i, TCH)],
                in1=rrep[:],
                op=MULT,
            )

        # ---- the attention stream ----
        eb_tiles = {}
        # ctx matmuls trail the score/exp stream by ~one slab, across tch
        # boundaries: entries (st, cps, pt_slab, slab_pos)
        ctx_q = deque()

        def eb_load(g):
            if g >= NTCH * NST:
                return
            tch, st = g // NST, g % NST
            eb = eb_pool.tile([128, HPC * TCH], F16, tag="eb", name="eb")
            nc.sync.dma_start(out=eb, in_=eb6[tch, st])
            eb_tiles[g] = eb

        def pop_ctx(n):
            for _ in range(n):
                if not ctx_q:
                    return
                st_, cps_, pt_, pos_ = ctx_q.popleft()
                if st_ == 0:
                    # zero rows 0..HD of both banks (sets has_written there)
                    # so the per-block accumulations can all use start=False
                    for bank in range(2):
                        nc.tensor.matmul(
                            cps_[0 : HD + 1, ts(bank, 512)],
                            lhsT=z_sb[:, 0 : HD + 1],
                            rhs=z_sb[:],
                            start=True,
                            stop=False,
                        )
                for i in range(HPC):
                    nc.tensor.matmul(
                        cps_[0 : HD + 1, ts(i, TCH)],
                        lhsT=v_tiles[st_][:, HB[i], :],
                        rhs=pt_[:, pos_ * 1024 + i * TCH : pos_ * 1024 + (i + 1) * TCH],
                        start=False,
                        stop=(st_ == NST - 1 and i % 2 == 1),
                    )

        def attention_tch(tch, interleave):
            # interleave: list of (st, fn); fn emitted just before that st
            cps = ctx_ps.tile(
                [128, HPC * TCH], F32, tag=f"cps{tch % 2}", name=f"cps{tch % 2}"
            )
            pending = deque(sorted(interleave, key=lambda e: e[0]))
            slab = {}

            for st in range(NST):
                while pending and pending[0][0] <= st:
                    pending.popleft()[1]()
                eb_load(tch * NST + st + EB_PF)
                if st % SLAB == 0:
                    slab["pre"] = pre_pool.tile(
                        [128, SLAB * 1024], F16, tag="pre", name="pre"
                    )
                sc = ps_pool.tile([128, 1024], F32, tag="sc", name="sc")
                with tc.high_priority(offset=400):
                    for hp in range(2):
                        mms = []
                        for j in range(2):
                            # block 2j+hp: the j-pair hits different banks
                            mm = nc.tensor.matmul(
                                sc[:, ts(2 * j + hp, TCH)],
                                lhsT=KT_sb[ts(j, HD), hp, ts(st, 128)],
                                rhs=QT_sb[ts(j, HD), hp, tch * TCH : tch * TCH + TCH],
                                start=True,
                                stop=True,
                            )
                            mms.append(mm)
                        add_dep_helper(
                            mms[1].ins, mms[0].ins, sync=False,
                            reason="score pair adjacency",
                        )
                eb = eb_tiles.pop(tch * NST + st)
                nc.vector.tensor_tensor(
                    out=slab["pre"][:, (st % SLAB) * 1024 : (st % SLAB + 1) * 1024],
                    in0=sc[:],
                    in1=eb[:],
                    op=ADD,
                )
                if st % SLAB == SLAB - 1:
                    pt = pt_pool.tile([128, SLAB * 1024], BF, tag="pt", name="pt")
                    nc.scalar.activation(out=pt, in_=slab["pre"], func=EXP)
                    for i in range(SLAB):
                        ctx_q.append((st - SLAB + 1 + i, cps, pt, i))
                    pop_ctx(2)  # drain slightly faster than fill to bound lag
                else:
                    pop_ctx(1)
            return cps

        # ---- emission ----
        # prologue: chunk-interleaved first loads so proj matmul c can start
        # as soon as weight/activation chunk c lands
        qt0 = ld_pool.tile([128, DCH, 512], BF, tag="qld", name="qt_sb")
        kt0 = ld_pool.tile([128, DCH, 512], BF, tag="kld", name="kt_sb")
        for c in range(DCH):
            nc.sync.dma_start(out=wq_sb[:, c, :], in_=wq5[:, c, :])
            nc.sync.dma_start(out=qt0[:, c, :], in_=q5[0, :, c, :])
        for c in range(DCH):
            nc.sync.dma_start(out=wk_sb[:, c, :], in_=wk5[:, c, :])
            nc.sync.dma_start(out=kt0[:, c, :], in_=k5[0, :, c, :])
        for g in range(EB_PF):
            eb_load(g)
        load_late_consts()
        proj_q_half(qt0, 0, 0)
        proj_k_half(kt0, 0, 0)
        # v-tile st must be emitted before its ctx pop; with pop pacing
        # (2 per slab-end slot, 1 otherwise) ctx(st) pops at slot >= st+3
        il0 = [
            (0, lambda: proj_q_half(qt0, 0, 1)),
            (0, lambda: proj_k_half(kt0, 0, 1)),
        ]
        k1a, k1b = make_k_parts(1)
        k2a, k2b = make_k_parts(2)
        k3a, k3b = make_k_parts(3)
        # KT chunk n covers st 4n..4n+3, needed at score slot 4n
        il0 += [(2, k1a), (2, k1b), (5, k2a), (6, k2b), (9, k3a), (10, k3b)]
        vslots = [1, 1, 3, 4, 4, 5, 6, 7, 7, 8, 11, 12, 12, 13, 14, 15]
        il0 += [
            (vslots[st], (lambda s: lambda: proj_v_tile(s))(st)) for st in range(NST)
        ]
        cps_prev = attention_tch(0, il0)
        for tch in range(1, NTCH):
            il = []
            # normalize the previous tch's heads once its ctx has drained
            # (pops of its last sts happen in slots 0..3 of this tch)
            for i in range(HPC):
                il.append((6 + i, (lambda c, t, ii: lambda: norm_block(c, t, ii))(
                    cps_prev, tch - 1, i)))
            # out-proj of tch-1 after its norm completes
            for tt in range(2):
                il.append((11 + 3 * tt, (lambda t, x: lambda: out_proj_tile(t, x))(
                    tch - 1, tt)))
            # Q chunk tch2 covers tches 2*tch2, 2*tch2+1; emit one tch ahead
            if tch % 2 == 1 and tch < NTCH - 1:
                qa, qb = make_q_parts((tch + 1) // 2)
                il += [(1, qa), (13, qb)]
            cps_prev = attention_tch(tch, il)
        # tail: drain remaining ctx, then norm + out-proj of the last tch
        pop_ctx(NST)
        for i in range(HPC):
            norm_block(cps_prev, NTCH - 1, i)
        for tt in range(2):
            out_proj_tile(NTCH - 1, tt)

    nc.compile()
    return nc


def _get_program():
    global _PROGRAM
    if _PROGRAM is None:
        _PROGRAM = build_program()
    return _PROGRAM


def make_in_maps(query, key, value, attn_bias, Wq, bq, Wk, Wv, Wo):
    bf = ml_dtypes.bfloat16
    f32 = np.float32
    query = np.asarray(query, f32)
    key = np.asarray(key, f32)
    value = np.asarray(value, f32)
    attn_bias = np.asarray(attn_bias, f32)
    Wq, Wk, Wv, Wo = (np.asarray(w, f32) for w in (Wq, Wk, Wv, Wo))
    bq = np.asarray(bq, f32)
    in_maps = []
    for c in range(NCORES):
        b, hg = c // 4, c % 4
        dsl = slice(DPC * hg, DPC * (hg + 1))
        hsl = slice(HPC * hg, HPC * (hg + 1))
        # [p, c, t] layouts, contiguous per chunk
        q5 = np.ascontiguousarray(
            query[b].T.reshape(DCH, 128, 4, 512).transpose(2, 1, 0, 3)
        ).astype(bf)
        k5 = np.ascontiguousarray(
            key[b].T.reshape(DCH, 128, 4, 512).transpose(2, 1, 0, 3)
        ).astype(bf)
        v5 = np.ascontiguousarray(
            value[b].T.reshape(DCH, 128, NST, 128).transpose(2, 1, 0, 3)
        ).astype(bf)
        # eb6[tch, st, p, i*TCH+t'] = bias[b, 4hg+HB[i], tch*TCH+t', st*128+p]
        # (block order HB matches the on-device score block layout)
        eb6 = np.ascontiguousarray(
            attn_bias[b, hsl][HB]
            .reshape(HPC, NTCH, TCH, NST, 128)
            .transpose(1, 3, 4, 0, 2)
            .reshape(NTCH, NST, 128, HPC * TCH)
        ).astype(np.float16)
        wq5 = np.ascontiguousarray(
            Wq[dsl].T.reshape(DCH, 128, DPC).transpose(1, 0, 2)
        ).astype(bf)
        wk5 = np.ascontiguousarray(
            Wk[dsl].T.reshape(DCH, 128, DPC).transpose(1, 0, 2)
        ).astype(bf)
        wv5 = np.ascontiguousarray(
            Wv[dsl].T.reshape(DCH, 128, DPC).transpose(1, 0, 2)
        ).astype(bf)
        wo5 = np.ascontiguousarray(
            Wo[:, dsl].T.reshape(2, 128, D).transpose(1, 0, 2)
        ).astype(bf)
        in_maps.append(
            {
                "q5": q5,
                "k5": k5,
                "v5": v5,
                "eb6": eb6,
                "wq5": wq5,
                "wk5": wk5,
                "wv5": wv5,
                "wo5": wo5,
                "bq": np.ascontiguousarray(
                    (bq[dsl] / 8.0).reshape(2, 128, 1).transpose(1, 0, 2)
                ),
            }
        )
    return in_maps


def combine_outputs(results, Wo, bv, bo):
    out = np.zeros((B, T, D), np.float64)
    for c in range(NCORES):
        out[c // 4] += results[c]["outp"].astype(np.float64)
    const = np.asarray(bv, np.float64) @ np.asarray(Wo, np.float64).T + np.asarray(
        bo, np.float64
    )
    out += const
    return out.astype(np.float32)


def kernel(
    query,
    key,
    value,
    attn_bias,
    key_padding_mask,
    Wq,
    bq,
    Wk,
    bk,
    Wv,
    bv,
    Wo,
    bo,
):
    # key_padding_mask is all-False in this problem; bk is dropped (softmax is
    # invariant to a per-row constant shift); bv/bo enter via a host constant.
    nc = _get_program()
    in_maps = make_in_maps(query, key, value, attn_bias, Wq, bq, Wk, Wv, Wo)
    res = run_bass_kernel_spmd(nc, in_maps, list(range(NCORES)))
    return combine_outputs(res.results, Wo, bv, bo)


if __name__ == "__main__":
    rng = np.random.default_rng(0)
    args = {
        "query": rng.standard_normal((B, T, D), np.float32),
        "key": rng.standard_normal((B, S, D), np.float32),
        "value": rng.standard_normal((B, S, D), np.float32),
        "attn_bias": rng.standard_normal((B, H, T, S), np.float32),
        "key_padding_mask": np.zeros((B, S), bool),
        "Wq": rng.uniform(-0.03125, 0.03125, (D, D)).astype(np.float32),
        "bq": rng.uniform(-0.03125, 0.03125, D).astype(np.float32),
        "Wk": rng.uniform(-0.03125, 0.03125, (D, D)).astype(np.float32),
        "bk": rng.uniform(-0.03125, 0.03125, D).astype(np.float32),
        "Wv": rng.uniform(-0.03125, 0.03125, (D, D)).astype(np.float32),
        "bv": rng.uniform(-0.03125, 0.03125, D).astype(np.float32),
        "Wo": rng.uniform(-0.03125, 0.03125, (D, D)).astype(np.float32),
        "bo": rng.uniform(-0.03125, 0.03125, D).astype(np.float32),
    }
    out = kernel(**args)
    print("kernel ran, out shape", out.shape, "std", out.std())


# revision 16
# speedup vs baseline: 1.3579x; 1.3579x over previous
"""Multi-head attention (16 heads, d_model=1024, B=2, T=S=2048) on 8 trn2 cores.

Strategy v4: shard by (batch, head-quad) — core c handles batch c//4, heads
4*(c%4)..+4. Differences vs v3 (the 257us baseline):
  - attn_bias enters ADDITIVELY as raw fp16: the PSUM->SBUF drain of the
    score tile is one DVE scalar_tensor_tensor (sc*0.125 + bias) into an
    f32 SBUF slab, replacing the separate exp(bias) bf16 multiply.
  - exp runs as wide [128,4096] SBUF->SBUF activations (4 s-tiles per
    ACTIVATE) cutting ScalarE per-instruction overhead ~25%.
  - t-chunks of 256 (not 512) so the 4 per-head ctx accumulators fit in
    2 PSUM banks and can double-buffer across t-chunks: no end-of-chunk
    normalization stall, PE stays HAM-warm.
  - all DMA sources are host-relaid so every transfer is >=2KB/partition
    contiguous.
  - Q-projection drain runs on ScalarE (Identity+bias), normalization uses
    reciprocal straight from PSUM; denominators still ride the V ones-column.
Host sums the 4 partials per batch and adds bo + bv@Wo.T.
"""

import sys

sys.path.insert(0, "/opt/trn_rl_repo")

from collections import deque
from contextlib import ExitStack

import ml_dtypes
import numpy as np

from concourse import bacc, mybir
from concourse.bass import ts
from concourse.bass_utils import run_bass_kernel_spmd
from concourse.tile import TileContext
from concourse.tile_rust import add_dep_helper

B, T, S, D, H, HD = 2, 2048, 2048, 1024, 16, 64
NCORES = 8
HPC = 4  # heads per core
DPC = HPC * HD  # 256 head-dims per core
DCH = D // 128  # 8 dmodel chunks
NST = S // 128  # 16 s-tiles
TCH = 256  # t-chunk
NTCH = T // TCH  # 8 t-chunks
# column-block order within sc/cps/eb/pt tiles: block i holds head HB[i].
# j-major so the row-group-concurrent score pair (j=0,1) lands in different
# PSUM banks (block 2j+hp), and so ctx accumulation start/stop can follow
# first-in-bank/last-in-bank order (bank0 = blocks 0,1; bank1 = blocks 2,3).
HB = [0, 2, 1, 3]
SLAB = 4  # s-tiles per exp slab
EB_PF = 8  # eb DMA prefetch depth (slots)
BF = mybir.dt.bfloat16
F16 = mybir.dt.float16
F32 = mybir.dt.float32
EXP = mybir.ActivationFunctionType.Exp
IDENT = mybir.ActivationFunctionType.Identity
ADD = mybir.AluOpType.add
MULT = mybir.AluOpType.mult

_PROGRAM = None


def build_program():
    nc = bacc.Bacc()
    # host-relaid inputs; all DMAs are contiguous per partition
    q5 = nc.declare_dram_parameter("q5", [4, 128, DCH, 512], BF, isOutput=False)
    k5 = nc.declare_dram_parameter("k5", [4, 128, DCH, 512], BF, isOutput=False)
    v5 = nc.declare_dram_parameter("v5", [NST, 128, DCH, 128], BF, isOutput=False)
    # multiplicative bias: eb6[tch, st, p, i*TCH+t'] = exp(bias) bf16
    eb6 = nc.declare_dram_parameter(
        "eb6", [NTCH, NST, 128, HPC * TCH], BF, isOutput=False
    )
    wq5 = nc.declare_dram_parameter("wq5", [128, DCH, DPC], BF, isOutput=False)
    wk5 = nc.declare_dram_parameter("wk5", [128, DCH, DPC], BF, isOutput=False)
    wv5 = nc.declare_dram_parameter("wv5", [128, DCH, DPC], BF, isOutput=False)
    wo5 = nc.declare_dram_parameter("wo5", [128, 2, D], BF, isOutput=False)
    bq_d = nc.declare_dram_parameter("bq", [128, 2, 1], F32, isOutput=False)
    outp = nc.declare_dram_parameter("outp", [T, D], BF, isOutput=True)

    with TileContext(nc) as tc, ExitStack() as ctx:
        consts = ctx.enter_context(tc.tile_pool(name="consts", bufs=1))
        ld_pool = ctx.enter_context(tc.tile_pool(name="ld", bufs=2))
        qkv_pool = ctx.enter_context(tc.tile_pool(name="qkv", bufs=1))
        vsb_pool = ctx.enter_context(tc.tile_pool(name="vsb", bufs=1))
        eb_pool = ctx.enter_context(tc.tile_pool(name="eb", bufs=EB_PF + 2))
        pt_pool = ctx.enter_context(tc.tile_pool(name="pt", bufs=14))
        norm_pool = ctx.enter_context(tc.tile_pool(name="norm", bufs=4))
        outs_pool = ctx.enter_context(tc.tile_pool(name="outs", bufs=3))
        # PSUM: sc tag 2 bufs x 2 banks = 4, ctx 2 tags x 2 banks = 4 -> 8
        ps_pool = ctx.enter_context(tc.tile_pool(name="ps", bufs=2, space="PSUM"))
        ctx_ps = ctx.enter_context(tc.tile_pool(name="ctx_ps", bufs=1, space="PSUM"))

        # preload the exp table while initial DMAs stream
        warm = consts.tile([1, 8], F32, tag="warm")
        nc.vector.memset(warm[:], 0.0)
        nc.scalar.activation(out=warm[:], in_=warm[:], func=EXP)
        z_sb = consts.tile([128, 512], BF, tag="zeros")
        nc.vector.memset(z_sb[:], 0.0)

        # ---- constants ----
        wq_sb = consts.tile([128, DCH, DPC], BF, tag="wq")
        wk_sb = consts.tile([128, DCH, DPC], BF, tag="wk")
        wv_sb = consts.tile([128, DCH, DPC], BF, tag="wv")
        wo_sb = consts.tile([128, 2, D], BF, tag="wo")
        bq_sb = consts.tile([128, 2, 1], F32, tag="bq")
        nc.sync.dma_start(out=bq_sb, in_=bq_d[:])

        def load_late_consts():
            nc.sync.dma_start(out=wv_sb, in_=wv5[:])
            nc.sync.dma_start(out=wo_sb, in_=wo5[:])

        # persistent activations; partition p = j*64+d, axis1 = hp
        QT_sb = qkv_pool.tile([128, 2, T], BF, tag="QT")
        KT_sb = qkv_pool.tile([128, 2, S], BF, tag="KT")
        ctxT_sb = qkv_pool.tile([128, 2, T], BF, tag="ctxT")
        v_tiles = [
            vsb_pool.tile([128, HPC, HD + 1], BF, tag=f"v{st}", name=f"v{st}")
            for st in range(NST)
        ]

        def load_q_chunk(tch2):
            qt_sb = ld_pool.tile([128, DCH, 512], BF, tag="qld", name="qt_sb")
            nc.sync.dma_start(out=qt_sb, in_=q5[tch2])
            return qt_sb

        def load_k_chunk(tch2):
            kt_sb = ld_pool.tile([128, DCH, 512], BF, tag="kld", name="kt_sb")
            nc.sync.dma_start(out=kt_sb, in_=k5[tch2])
            return kt_sb

        def proj_q_half(qt_sb, tch2, half):
            pq = ps_pool.tile([128, 1024], F32, tag="sc", name="pq")
            for c in range(DCH):
                nc.tensor.matmul(
                    pq[:, 0:512],
                    lhsT=wq_sb[:, c, ts(half, 128)],
                    rhs=qt_sb[:, c, :],
                    start=(c == 0),
                    stop=(c == DCH - 1),
                )
            # QT = (Q + bq) / 8  (attention scale folded in)
            nc.vector.tensor_scalar(
                out=QT_sb[:, half, ts(tch2, 512)],
                in0=pq[:, 0:512],
                scalar1=bq_sb[:, half, :],
                scalar2=0.125,
                op0=ADD,
                op1=MULT,
            )

        def proj_k_half(kt_sb, tch2, half):
            pk = ps_pool.tile([128, 1024], F32, tag="sc", name="pk")
            for c in range(DCH):
                nc.tensor.matmul(
                    pk[:, 0:512],
                    lhsT=wk_sb[:, c, ts(half, 128)],
                    rhs=kt_sb[:, c, :],
                    start=(c == 0),
                    stop=(c == DCH - 1),
                )
            nc.vector.tensor_copy(
                out=KT_sb[:, half, ts(tch2, 512)], in_=pk[:, 0:512]
            )

        def make_q_parts(tch2):
            state = {}

            def part0():
                state["qt"] = load_q_chunk(tch2)
                proj_q_half(state["qt"], tch2, 0)

            def part1():
                proj_q_half(state["qt"], tch2, 1)

            return part0, part1

        def make_k_parts(tch2):
            state = {}

            def part0():
                state["kt"] = load_k_chunk(tch2)
                proj_k_half(state["kt"], tch2, 0)

            def part1():
                proj_k_half(state["kt"], tch2, 1)

            return part0, part1

        def proj_v_tile(st):
            vt_sb = ld_pool.tile([128, DCH, 128], BF, tag="vld", name="vt_sb", bufs=3)
            nc.sync.dma_start(out=vt_sb, in_=v5[st])
            pv = ps_pool.tile([128, 1024], F32, tag="sc", name="pv")
            for c in range(DCH):
                nc.tensor.matmul(
                    pv[:, 0:DPC],
                    lhsT=vt_sb[:, c, :],
                    rhs=wv_sb[:, c, :],
                    start=(c == 0),
                    stop=(c == DCH - 1),
                )
            v_sb = v_tiles[st]
            nc.vector.tensor_copy(
                out=v_sb[:, :, 0:HD],
                in_=pv[:, 0:DPC].rearrange("p (h d) -> p h d", h=HPC),
            )
            nc.vector.memset(v_sb[:, :, HD : HD + 1], 1.0)

        def out_proj_tile(tch, tt):
            # out rows [tch*TCH + tt*128, +128); emitted during the NEXT tch
            t0 = tch * TCH + tt * 128
            po = ps_pool.tile([128, 1024], F32, tag="sc", name="po")
            for eh in range(2):
                for half in range(2):
                    nc.tensor.matmul(
                        po[:, ts(eh, 512)],
                        lhsT=ctxT_sb[:, half, t0 : t0 + 128],
                        rhs=wo_sb[:, half, ts(eh, 512)],
                        start=(half == 0),
                        stop=(half == 1),
                    )
            out_sb = outs_pool.tile([128, D], BF, tag="out", name="out_sb")
            nc.vector.tensor_copy(out=out_sb, in_=po[:])
            nc.sync.dma_start(out=outp[t0 : t0 + 128, :], in_=out_sb)

        def norm_pre(cps):
            # denominators live in row HD; one copy + one reciprocal per tch
            dn = norm_pool.tile([1, HPC * TCH], F32, tag="dn", name="dn", bufs=2)
            nc.vector.tensor_copy(out=dn[:], in_=cps[HD : HD + 1, :])
            rc = norm_pool.tile([1, HPC * TCH], F32, tag="rc", name="rc", bufs=2)
            nc.vector.reciprocal_approx_fast(out=rc[:], in_=dn[:])
            return rc

        def norm_block(cps, rc, tch, i):
            # normalize block i (head HB[i]) into ctxT
            h = HB[i]
            rrep = norm_pool.tile([64, TCH], F32, tag="rrep", name="rrep")
            nc.gpsimd.partition_broadcast(rrep[:], rc[0:1, ts(i, TCH)], channels=64)
            nc.vector.tensor_tensor(
                out=ctxT_sb[ts(h % 2, HD), h // 2, tch * TCH : tch * TCH + TCH],
                in0=cps[0:HD, ts(i, TCH)],
                in1=rrep[:],
                op=MULT,
            )

        # ---- the attention stream ----
        eb_tiles = {}
        # ctx matmuls trail the score/exp stream by ~one slab, across tch
        # boundaries: entries (st, cps, pt_slab, slab_pos)
        ctx_q = deque()

        def eb_load(g):
            if g >= NTCH * NST:
                return
            tch, st = g // NST, g % NST
            eb = eb_pool.tile([128, HPC * TCH], BF, tag="eb", name="eb")
            nc.sync.dma_start(out=eb, in_=eb6[tch, st])
            eb_tiles[g] = eb

        def pop_ctx(n):
            for _ in range(n):
                if not ctx_q:
                    return
                st_, cps_, pt_ = ctx_q.popleft()
                if st_ == 0:
                    # zero rows 0..HD of both banks (sets has_written there)
                    # so the per-block accumulations can all use start=False
                    for bank in range(2):
                        nc.tensor.matmul(
                            cps_[0 : HD + 1, ts(bank, 512)],
                            lhsT=z_sb[:, 0 : HD + 1],
                            rhs=z_sb[:],
                            start=True,
                            stop=False,
                        )
                for i in range(HPC):
                    nc.tensor.matmul(
                        cps_[0 : HD + 1, ts(i, TCH)],
                        lhsT=v_tiles[st_][:, HB[i], :],
                        rhs=pt_[:, ts(i, TCH)],
                        start=False,
                        stop=(st_ == NST - 1 and i % 2 == 1),
                    )

        def attention_tch(tch, interleave):
            # interleave: list of (st, fn); fn emitted just before that st
            cps = ctx_ps.tile(
                [128, HPC * TCH], F32, tag=f"cps{tch % 2}", name=f"cps{tch % 2}"
            )
            pending = deque(sorted(interleave, key=lambda e: e[0]))

            for st in range(NST):
                while pending and pending[0][0] <= st:
                    pending.popleft()[1]()
                eb_load(tch * NST + st + EB_PF)
                sc = ps_pool.tile([128, 1024], F32, tag="sc", name="sc")
                with tc.high_priority(offset=400):
                    for hp in range(2):
                        mms = []
                        for j in range(2):
                            # block 2j+hp: the j-pair hits different banks
                            mm = nc.tensor.matmul(
                                sc[:, ts(2 * j + hp, TCH)],
                                lhsT=KT_sb[ts(j, HD), hp, ts(st, 128)],
                                rhs=QT_sb[ts(j, HD), hp, tch * TCH : tch * TCH + TCH],
                                start=True,
                                stop=True,
                            )
                            mms.append(mm)
                        add_dep_helper(
                            mms[1].ins, mms[0].ins, sync=False,
                            reason="score pair adjacency",
                        )
                pt = pt_pool.tile([128, 1024], BF, tag="pt", name="pt")
                nc.scalar.activation(out=pt[:], in_=sc[:], func=EXP)
                eb = eb_tiles.pop(tch * NST + st)
                nc.vector.tensor_tensor(out=pt[:], in0=pt[:], in1=eb[:], op=MULT)
                ctx_q.append((st, cps, pt))
                # delayed pops: tch0 defers ctx 12 slots so the v-tile/weight
                # DMA storm drains first; steady state keeps a ~4-st lag
                if tch == 0 and st < NST - SLAB:
                    pass
                elif len(ctx_q) > 5:
                    pop_ctx(2)
                elif len(ctx_q) > 3:
                    pop_ctx(1)
            return cps

        # ---- emission ----
        # prologue: chunk-interleaved first loads so proj matmul c can start
        # as soon as weight/activation chunk c lands
        qt0 = ld_pool.tile([128, DCH, 512], BF, tag="qld", name="qt_sb")
        kt0 = ld_pool.tile([128, DCH, 512], BF, tag="kld", name="kt_sb")
        for c in range(DCH):
            nc.sync.dma_start(out=wq_sb[:, c, :], in_=wq5[:, c, :])
            nc.sync.dma_start(out=qt0[:, c, :], in_=q5[0, :, c, :])
        for c in range(DCH):
            nc.sync.dma_start(out=wk_sb[:, c, :], in_=wk5[:, c, :])
            nc.sync.dma_start(out=kt0[:, c, :], in_=k5[0, :, c, :])
        for g in range(EB_PF):
            eb_load(g)
        load_late_consts()
        proj_q_half(qt0, 0, 0)
        proj_k_half(kt0, 0, 0)
        # v-tile st must be emitted before its ctx pop; with pop pacing
        # (2 per slab-end slot, 1 otherwise) ctx(st) pops at slot >= st+3
        il0 = [
            (0, lambda: proj_q_half(qt0, 0, 1)),
            (0, lambda: proj_k_half(kt0, 0, 1)),
        ]
        k1a, k1b = make_k_parts(1)
        k2a, k2b = make_k_parts(2)
        k3a, k3b = make_k_parts(3)
        # KT chunk n covers st 4n..4n+3, needed at score slot 4n
        il0 += [(2, k1a), (2, k1b), (5, k2a), (6, k2b), (9, k3a), (10, k3b)]
        # v tiles are first consumed at tch0 slot 12 (delayed pops); spread
        # their DMA evenly, with the last 4 landing early in tch1
        il0 += [
            (4 + st, (lambda s: lambda: proj_v_tile(s))(st)) for st in range(12)
        ]
        cps_prev = attention_tch(0, il0)
        rc_state = {}
        for tch in range(1, NTCH):
            il = []
            if tch == 1:
                il += [
                    (st - 12, (lambda s: lambda: proj_v_tile(s))(st))
                    for st in range(12, NST)
                ]
            # normalize the previous tch once its ctx has drained (pops of
            # its last sts happen in slots 0..3 of this tch)
            il.append((6, (lambda c: lambda: rc_state.__setitem__("rc", norm_pre(c)))(
                cps_prev)))
            for i in range(HPC):
                il.append((7 + i, (lambda c, t, ii: lambda: norm_block(
                    c, rc_state["rc"], t, ii))(cps_prev, tch - 1, i)))
            # out-proj of tch-1 after its norm completes
            for tt in range(2):
                il.append((12 + 3 * tt, (lambda t, x: lambda: out_proj_tile(t, x))(
                    tch - 1, tt)))
            # Q chunk tch2 covers tches 2*tch2, 2*tch2+1; emit one tch ahead
            if tch % 2 == 1 and tch < NTCH - 1:
                qa, qb = make_q_parts((tch + 1) // 2)
                il += [(2, qa), (13, qb)]
            cps_prev = attention_tch(tch, il)
        # tail: drain remaining ctx, then norm + out-proj of the last tch
        pop_ctx(NST)
        rc = norm_pre(cps_prev)
        for i in range(HPC):
            norm_block(cps_prev, rc, NTCH - 1, i)
        for tt in range(2):
            out_proj_tile(NTCH - 1, tt)

    nc.compile()
    return nc


def _get_program():
    global _PROGRAM
    if _PROGRAM is None:
        _PROGRAM = build_program()
    return _PROGRAM


def make_in_maps(query, key, value, attn_bias, Wq, bq, Wk, Wv, Wo):
    bf = ml_dtypes.bfloat16
    f32 = np.float32
    query = np.asarray(query, f32)
    key = np.asarray(key, f32)
    value = np.asarray(value, f32)
    attn_bias = np.asarray(attn_bias, f32)
    Wq, Wk, Wv, Wo = (np.asarray(w, f32) for w in (Wq, Wk, Wv, Wo))
    bq = np.asarray(bq, f32)
    in_maps = []
    for c in range(NCORES):
        b, hg = c // 4, c % 4
        dsl = slice(DPC * hg, DPC * (hg + 1))
        hsl = slice(HPC * hg, HPC * (hg + 1))
        # [p, c, t] layouts, contiguous per chunk
        q5 = np.ascontiguousarray(
            query[b].T.reshape(DCH, 128, 4, 512).transpose(2, 1, 0, 3)
        ).astype(bf)
        k5 = np.ascontiguousarray(
            key[b].T.reshape(DCH, 128, 4, 512).transpose(2, 1, 0, 3)
        ).astype(bf)
        v5 = np.ascontiguousarray(
            value[b].T.reshape(DCH, 128, NST, 128).transpose(2, 1, 0, 3)
        ).astype(bf)
        # eb6[tch, st, p, i*TCH+t'] = exp(bias[b, 4hg+HB[i], tch*TCH+t',
        # st*128+p]) -- block order HB matches the on-device score layout
        eb6 = np.ascontiguousarray(
            np.exp(attn_bias[b, hsl][HB])
            .reshape(HPC, NTCH, TCH, NST, 128)
            .transpose(1, 3, 4, 0, 2)
            .reshape(NTCH, NST, 128, HPC * TCH)
        ).astype(bf)
        wq5 = np.ascontiguousarray(
            Wq[dsl].T.reshape(DCH, 128, DPC).transpose(1, 0, 2)
        ).astype(bf)
        wk5 = np.ascontiguousarray(
            Wk[dsl].T.reshape(DCH, 128, DPC).transpose(1, 0, 2)
        ).astype(bf)
        wv5 = np.ascontiguousarray(
            Wv[dsl].T.reshape(DCH, 128, DPC).transpose(1, 0, 2)
        ).astype(bf)
        wo5 = np.ascontiguousarray(
            Wo[:, dsl].T.reshape(2, 128, D).transpose(1, 0, 2)
        ).astype(bf)
        in_maps.append(
            {
                "q5": q5,
                "k5": k5,
                "v5": v5,
                "eb6": eb6,
                "wq5": wq5,
                "wk5": wk5,
                "wv5": wv5,
                "wo5": wo5,
                "bq": np.ascontiguousarray(
                    bq[dsl].reshape(2, 128, 1).transpose(1, 0, 2)
                ),
            }
        )
    return in_maps


def combine_outputs(results, Wo, bv, bo):
    out = np.zeros((B, T, D), np.float64)
    for c in range(NCORES):
        out[c // 4] += results[c]["outp"].astype(np.float64)
    const = np.asarray(bv, np.float64) @ np.asarray(Wo, np.float64).T + np.asarray(
        bo, np.float64
    )
    out += const
    return out.astype(np.float32)


def kernel(
    query,
    key,
    value,
    attn_bias,
    key_padding_mask,
    Wq,
    bq,
    Wk,
    bk,
    Wv,
    bv,
    Wo,
    bo,
):
    # key_padding_mask is all-False in this problem; bk is dropped (softmax is
    # invariant to a per-row constant shift); bv/bo enter via a host constant.
    nc = _get_program()
    in_maps = make_in_maps(query, key, value, attn_bias, Wq, bq, Wk, Wv, Wo)
    res = run_bass_kernel_spmd(nc, in_maps, list(range(NCORES)))
    return combine_outputs(res.results, Wo, bv, bo)


if __name__ == "__main__":
    rng = np.random.default_rng(0)
    args = {
        "query": rng.standard_normal((B, T, D), np.float32),
        "key": rng.standard_normal((B, S, D), np.float32),
        "value": rng.standard_normal((B, S, D), np.float32),
        "attn_bias": rng.standard_normal((B, H, T, S), np.float32),
        "key_padding_mask": np.zeros((B, S), bool),
        "Wq": rng.uniform(-0.03125, 0.03125, (D, D)).astype(np.float32),
        "bq": rng.uniform(-0.03125, 0.03125, D).astype(np.float32),
        "Wk": rng.uniform(-0.03125, 0.03125, (D, D)).astype(np.float32),
        "bk": rng.uniform(-0.03125, 0.03125, D).astype(np.float32),
        "Wv": rng.uniform(-0.03125, 0.03125, (D, D)).astype(np.float32),
        "bv": rng.uniform(-0.03125, 0.03125, D).astype(np.float32),
        "Wo": rng.uniform(-0.03125, 0.03125, (D, D)).astype(np.float32),
        "bo": rng.uniform(-0.03125, 0.03125, D).astype(np.float32),
    }
    out = kernel(**args)
    print("kernel ran, out shape", out.shape, "std", out.std())
